# revision 3
# baseline (speedup 1.0000x reference)
"""Trainium2 Bass kernel for nn_Attention_44074954391673 — v2.

Sharding: 8 cores; core c -> batch b = c//4, heads [3*(c%4), 3*(c%4)+3).

v2 changes vs baseline:
- t-outer/h-inner main loop: struct[t] loaded ONCE per tile (was 3x per head).
- Natural score scale (no /8 foldings); exp uses scale=1/8.
- fp8(e4m3) scratch for the rel-position windowed-matmul round-trip: halves
  scratch-write + readback DMA traffic.
- Batched output: per-head ctx written to an SBUF f32 buffer, one DMA per tile.
- Product-engine knobs: the 5 (bias_i)*struct_i units are split across
  DVE/Act/Pool; join-vs-tree split tunable.
"""
import os
import sys

sys.path.insert(0, "/opt/trn_rl_repo")

from contextlib import ExitStack

import numpy as np
import ml_dtypes

import concourse.bass as bass
import concourse.mybir as mybir
import concourse.tile as tile
from concourse import bacc
from concourse.bass import ds
from concourse.bass_utils import run_bass_kernel_spmd

BF16 = mybir.dt.bfloat16
FP8 = mybir.dt.float8e4
F32 = mybir.dt.float32
AF = mybir.ActivationFunctionType
OP = mybir.AluOpType

H, DH = 12, 64
B, L, D = 2, 1024, 768
NCHUNK = 6
WIN = 1152
NT = 8

# ---- tuning knobs ----
SCRATCH_DT = FP8      # dtype of rel-pos scratch in DRAM
PROD = "BDDDD"        # product engine i=0..4: D=DVE one-pass,
                      #   A=Act-evac+DVE-mult, B=Act-evac+Pool-mult
                      # (B first: its 3-hop PE->Act->Pool chain needs slack)
NJOIN = 5             # products joined into psc via PE identity (rest: DVE/Pool tree + 1 join)
WEVAC = "ADADAD"      # window-evac engine per chunk: A=Act, D=DVE (Pool can't read PSUM)
ST_PF = 2             # struct tile prefetch depth
RB_PF = 4             # r1/g readback prefetch depth (units ahead)


def build_program(use_mask: bool, use_pbias: bool, reps: int = 1, use_absb: bool = False):
    nc = bacc.Bacc("TRN2", target_bir_lowering=False, debug=False, num_devices=8)

    def din(name, shape, dt=BF16):
        return nc.dram_tensor(name, shape, dt, kind="ExternalInput").ap()

    hsT = din("hsT", [128, NCHUNK, 1024])
    wq = din("wq", [128, NCHUNK, 192])
    wk = din("wk", [128, NCHUNK, 192])
    wv = din("wv", [128, NCHUNK, 192])
    ert = din("ert", [32, 2, 2048], FP8)   # 16x-scaled, DoubleRow layout
    et = din("et", [32, 2, 2048], FP8)
    ssw = din("ssw", [64, 5, 3, 64])
    struct = din("struct", [NT, 128, 5, 1024])
    absb = din("absb", [1, 16], F32)
    idb = din("idb", [128, 128])
    idbs = din("idbs", [128, 128])         # eye / 16 (undoes table scaling)
    if use_mask:
        maskv = din("maskv", [1, 1024])
        onesv = din("onesv", [1, 128])
    if use_pbias:
        bqv = din("bqv", [1, 192])
        bkv = din("bkv", [1, 192])
        bvv = din("bvv", [1, 192])
        onesL = din("onesL", [1, 1024])
    out = nc.dram_tensor("out", [NT, 128, 192], F32, kind="ExternalOutput").ap()

    with tile.TileContext(nc) as tc:
        for _rep in range(reps):
          with ExitStack() as ctx:
            # ---------------- constants ----------------
            consts = ctx.enter_context(tc.tile_pool(name="consts", bufs=1))

            def cload(ap_in, shape=None, dt=BF16, name=None, pool=None):
                t = (pool or consts).tile(shape, dt, name=name)
                nc.sync.dma_start(t, ap_in)
                return t

            sb_wk = cload(wk, name="wk", shape=[128, NCHUNK, 192])
            sb_wq = cload(wq, name="wq", shape=[128, NCHUNK, 192])
            sb_wv = cload(wv, name="wv", shape=[128, NCHUNK, 192])
            sb_et = cload(et, name="et", shape=[32, 2, 2048], dt=FP8)
            sb_ert = cload(ert, name="ert", shape=[32, 2, 2048], dt=FP8)
            sb_idb = cload(idb, name="idb", shape=[128, 128])
            sb_idbs = cload(idbs, name="idbs", shape=[128, 128])
            sb_ssw = cload(ssw, name="ssw", shape=[64, 5, 3, 64])
            if use_absb:
                sb_absb = cload(
                    bass.AP(tensor=absb.tensor, offset=0, ap=[[0, 128], [1, 16]]),
                    name="absb_sb", shape=[128, 16], dt=F32,
                )
            if use_mask:
                sb_mask = cload(maskv, name="maskv", shape=[1, 1024])
                sb_ones = cload(onesv, name="onesv", shape=[1, 128])
            if use_pbias:
                sb_bq = cload(bqv, name="bqv", shape=[1, 192])
                sb_bk = cload(bkv, name="bkv", shape=[1, 192])
                sb_bv = cload(bvv, name="bvv", shape=[1, 192])
                sb_onesL = cload(onesL, name="onesL", shape=[1, 1024])

            qkv = ctx.enter_context(tc.tile_pool(name="qkv", bufs=1))
            qd = [qkv.tile([128, 1024], BF16, tag=f"qd{h}", name=f"qd{h}") for h in range(3)]
            kd = [qkv.tile([128, 1024], BF16, tag=f"kd{h}", name=f"kd{h}") for h in range(3)]
            vsb = qkv.tile([128, NT, 192], BF16, name="vsb")
            osb = qkv.tile([128, NT, 192], F32, name="osb")

            # struct tiles: rotating pool, loaded once per t, reused by 3 heads
            sstp = ctx.enter_context(tc.tile_pool(name="sstp", bufs=3))

            def st_load(t):
                st = sstp.tile([128, 5, 1024], BF16, tag="st", name=f"st{t}")
                nc.sync.dma_start(st, struct[t])
                return st

            st_tiles = {}

            dpool = ctx.enter_context(tc.tile_pool(name="dscr", bufs=1, space="DRAM"))
            dramQ = [dpool.tile([NT, 128, WIN], SCRATCH_DT, tag=f"dq{h}", name=f"dq{h}")
                     for h in range(3)]
            dramK = [dpool.tile([NT, 128, WIN], SCRATCH_DT, tag=f"dk{h}", name=f"dk{h}")
                     for h in range(3)]

            # ---------------- pools (windows share the putil psum ring) ----------
            qwp = ctx.enter_context(tc.tile_pool(name="qwp", bufs=1))
            qw_sb = [qwp.tile([128, 5, 1024], BF16, tag=f"qw{h}", name=f"qw{h}")
                     for h in range(3)]
            wev = ctx.enter_context(tc.tile_pool(name="wev", bufs=3))
            rg = ctx.enter_context(tc.tile_pool(name="rg", bufs=RB_PF + 2))
            bep = ctx.enter_context(tc.tile_pool(name="bep", bufs=2))
            prp = ctx.enter_context(tc.tile_pool(name="prp", bufs=2))
            prob = ctx.enter_context(tc.tile_pool(name="prob", bufs=3))
            misc = ctx.enter_context(tc.tile_pool(name="misc", bufs=3))

            wev_ctr = [0]

            putil = None  # created after the prologue (psum banks shared w/ pwp)

            # fp8 DoubleRow copies of q/k for the window matmuls
            f8p = ctx.enter_context(tc.tile_pool(name="f8p", bufs=1))
            qf8 = [f8p.tile([32, 2, 1024], FP8, tag=f"qf8{h}", name=f"qf8{h}")
                   for h in range(3)]
            kf8 = [f8p.tile([32, 2, 1024], FP8, tag=f"kf8{h}", name=f"kf8{h}")
                   for h in range(3)]

            def emit_f8(src, dst8, tmp):
                nc.gpsimd.tensor_copy(tmp, src[0:64, :])
                nc.sync.dma_start(dst8[:, 0, :], tmp[0:32, :])
                nc.sync.dma_start(dst8[:, 1, :], tmp[32:64, :])

            def emit_window(h, side, t0, pwpool=None):
                src = qf8[h] if side == 0 else kf8[h]
                rhs_tab = sb_ert if side == 0 else sb_et
                dst = dramQ[h] if side == 0 else dramK[h]
                # one wev tile holds the t0/t0+1 pair -> single paired DMA
                ev = wev.tile([128, 2, WIN], SCRATCH_DT, tag="wev", name="wev")
                for pi, tt in ((0, t0), (1, t0 + 1)):
                    win = 896 - 128 * tt
                    for ci, (c0, w) in enumerate(((0, 512), (512, 512), (1024, 128))):
                        pw = (pwpool or putil).tile(
                            [128, 512], F32,
                            tag="u" if pwpool is None else "pw", name="pw")
                        nc.tensor.matmul(
                            pw[:, 0:w],
                            lhsT=src[:, :, ds(128 * tt, 128)],
                            rhs=rhs_tab[:, :, ds(win + c0, w)],
                            start=True, stop=True,
                            perf_mode=mybir.MatmulPerfMode.DoubleRow,
                        )
                        e = WEVAC[wev_ctr[0] % len(WEVAC)]
                        wev_ctr[0] += 1
                        if e == "A":
                            nc.scalar.activation(ev[:, pi, ds(c0, w)], pw[:, 0:w], AF.Copy)
                        else:
                            nc.vector.tensor_copy(ev[:, pi, ds(c0, w)], pw[:, 0:w])
                nc.scalar.dma_start(dst[t0], ev[:, 0, :])
                nc.scalar.dma_start(dst[t0 + 1], ev[:, 1, :])

            def emit_qw(h, pwpool):
                for i in range(5):
                    for n2 in range(2):
                        pq = pwpool.tile([128, 512], F32, tag="pw", name="pq")
                        nc.tensor.matmul(
                            pq[0:64, :],
                            lhsT=sb_ssw[:, i, h, :],
                            rhs=qd[h][0:64, ds(512 * n2, 512)],
                            start=True, stop=True,
                        )
                        if (i + n2) % 2 == 0:
                            nc.scalar.activation(
                                qw_sb[h][0:64, i, ds(512 * n2, 512)], pq[0:64, :], AF.Copy)
                        else:
                            nc.vector.tensor_copy(
                                qw_sb[h][0:64, i, ds(512 * n2, 512)], pq[0:64, :])
                for i in range(5):
                    nc.sync.dma_start(qw_sb[h][64:128, i, :], qw_sb[h][0:64, i, :])

            # ---------------- prologue ----------------
            # k-projection first, then k-windows interleaved with q/v projection
            # pieces (PE filler while window evacs drain), then qw + first
            # q-window pair.  Remaining q-window pairs are woven into the loop.
            with tc.tile_pool(name="pp", bufs=2, space="PSUM") as pp, \
                 tc.tile_pool(name="ppb", bufs=1, space="PSUM") as ppb, \
                 tc.tile_pool(name="ptmp", bufs=1) as ptmp, \
                 tc.tile_pool(name="pwp", bufs=4, space="PSUM") as pwp:
                sb_hsc = [
                    cload(hsT[:, c, :], name=f"hsT{c}", shape=[128, 1024], pool=ptmp)
                    for c in range(NCHUNK)
                ]
                vta = ptmp.tile([128, 1024], BF16, tag="vta")
                vtb = ptmp.tile([64, 1024], BF16, tag="vtb")

                def proj_mms(w_sb, bias_sb, mlo, msz, n):
                    ps = pp.tile([128, 512], F32, tag="proj")
                    for c in range(NCHUNK):
                        last = (c == NCHUNK - 1) and not use_pbias
                        nc.tensor.matmul(
                            ps[0:msz, :],
                            lhsT=w_sb[:, c, ds(mlo, msz)],
                            rhs=sb_hsc[c][:, ds(512 * n, 512)],
                            start=(c == 0), stop=last,
                        )
                    if use_pbias:
                        nc.tensor.matmul(
                            ps[0:msz, :],
                            lhsT=bias_sb[0:1, ds(mlo, msz)],
                            rhs=sb_onesL[0:1, ds(512 * n, 512)],
                            start=False, stop=True,
                        )
                    return ps

                def proj_ab(w_sb, bias_sb, dup, n):
                    sl = ds(512 * n, 512)
                    ps = proj_mms(w_sb, bias_sb, 0, 128, n)
                    nc.scalar.activation(dup[0][0:64, sl], ps[0:64, :], AF.Copy)
                    nc.scalar.activation(dup[1][64:128, sl], ps[64:128, :], AF.Copy)
                    ps = proj_mms(w_sb, bias_sb, 128, 64, n)
                    nc.scalar.activation(dup[2][0:64, sl], ps[0:64, :], AF.Copy)

                def proj_dup(dup):
                    for h, (src, dst) in enumerate(((0, 64), (64, 0), (0, 64))):
                        nc.sync.dma_start(
                            dup[h][dst:dst + 64, :], dup[h][src:src + 64, :])

                # k first: its windows gate the whole t-loop
                kb = sb_bk if use_pbias else None
                qb = sb_bq if use_pbias else None
                vb = sb_bv if use_pbias else None
                for n in range(2):
                    proj_ab(sb_wk, kb, kd, n)
                proj_dup(kd)
                for h in range(3):
                    t8 = ptmp.tile([64, 1024], FP8, tag=f"t8k{h}", name="t8")
                    emit_f8(kd[h], kf8[h], t8)

                # filler pieces: q projection, q dup + fp8, v projection,
                # v transposes
                pieces = []
                for n in range(2):
                    pieces.append(lambda n=n: proj_ab(sb_wq, qb, qd, n))

                def qfin():
                    proj_dup(qd)
                    for h in range(3):
                        t8 = ptmp.tile([64, 1024], FP8, tag=f"t8q{h}", name="t8")
                        emit_f8(qd[h], qf8[h], t8)
                pieces.append(qfin)

                def vproj(n):
                    sl = ds(512 * n, 512)
                    ps = proj_mms(sb_wv, vb, 0, 128, n)
                    nc.scalar.activation(vta[:, sl], ps, AF.Copy)
                    ps = proj_mms(sb_wv, vb, 128, 64, n)
                    nc.scalar.activation(vtb[:, sl], ps[0:64, :], AF.Copy)
                pieces.append(lambda: vproj(0))
                pieces.append(lambda: vproj(1))

                def vt(t):
                    pst = ppb.tile([128, 128], BF16, tag="vtp")
                    nc.tensor.matmul(
                        pst, lhsT=vta[:, ds(128 * t, 128)], rhs=sb_idb,
                        is_transpose=True, start=True, stop=True,
                    )
                    nc.scalar.activation(vsb[:, t, 0:128], pst, AF.Copy)
                    pst2 = ppb.tile([128, 64], BF16, tag="vtp2")
                    nc.tensor.matmul(
                        pst2, lhsT=vtb[:, ds(128 * t, 128)], rhs=sb_idb[0:64, 0:64],
                        is_transpose=True, start=True, stop=True,
                    )
                    nc.scalar.activation(vsb[:, t, 128:192], pst2, AF.Copy)
                for t in range(NT):
                    pieces.append(lambda t=t: vt(t))

                for t in range(ST_PF):
                    st_tiles[t] = st_load(t)
                kwins = [(h, 1, t0) for h in range(3) for t0 in range(0, NT, 2)]
                pi_, np_ = 0, len(pieces)
                for wi, kw in enumerate(kwins):
                    emit_window(*kw, pwpool=pwp)
                    want = (wi + 1) * np_ // len(kwins)
                    while pi_ < want:
                        pieces[pi_]()
                        pi_ += 1
                while pi_ < np_:
                    pieces[pi_]()
                    pi_ += 1

                for h in range(3):
                    emit_qw(h, pwp)
                for t0 in (0, 2):
                    for h in range(3):
                        emit_window(h, 0, t0, pwpool=pwp)
            weave = []
            for t0 in (4, 6):
                for h in range(3):
                    weave.append((h, 0, t0))

            putil = ctx.enter_context(tc.tile_pool(name="putil", bufs=4, space="PSUM"))
            psS = ctx.enter_context(tc.tile_pool(name="psS", bufs=2, space="PSUM"))

            state = {}

            def stageDMA(u):
                t, h = divmod(u, 3)
                if h == 0 and t + ST_PF < NT:
                    st_tiles[t + ST_PF] = st_load(t + ST_PF)
                r1 = rg.tile([128, 1024], SCRATCH_DT, tag="r1", name="r1")
                nc.sync.dma_start(
                    r1,
                    bass.AP(
                        tensor=dramQ[h].tensor,
                        offset=dramQ[h].offset + t * 128 * WIN + 127,
                        ap=[[WIN - 1, 128], [1, 1024]],
                    ),
                )
                g = rg.tile([128, NT, 128], SCRATCH_DT, tag="g", name="g")
                nc.sync.dma_start(
                    g,
                    bass.AP(
                        tensor=dramK[h].tensor,
                        offset=dramK[h].offset + 128 * t + 127,
                        ap=[[WIN - 1, 128], [128 * WIN, NT], [1, 128]],
                    ),
                )
                state[u] = {"r1": r1, "g": g}

            def stageA(u):
                t, h = divmod(u, 3)
                qT, kT = qd[h], kd[h]
                psc = psS.tile([128, 1024], F32, tag="s", name="psc")
                nc.tensor.matmul(
                    psc[:, 0:512],
                    lhsT=qT[0:64, ds(128 * t, 128)], rhs=kT[0:64, 0:512],
                    start=True, stop=False, tile_position=(0, 0),
                )
                nc.tensor.matmul(
                    psc[:, 512:1024],
                    lhsT=qT[64:128, ds(128 * t, 128)],
                    rhs=kT[64:128, 512:1024],
                    start=True, stop=False, tile_position=(64, 0),
                )
                if use_mask:
                    for half in range(2):
                        sl = ds(512 * half, 512)
                        nc.tensor.matmul(
                            psc[:, sl], lhsT=sb_ones[0:1, :],
                            rhs=sb_mask[0:1, sl], start=False, stop=False,
                        )
                state[u]["psc"] = psc

            def stageB(u):
                t, h = divmod(u, 3)
                d = state[u]
                st = st_tiles[t]
                g, psc = d["g"], d["psc"]
                # k-side rel: transpose g chunks straight into the score psum
                # via regular matmuls with an identity ifmap.  Interleaved with
                # the biasmm stream so PE has fill work while pb slots drain.
                gq = [(j,) for j in range(NT)]

                def g_chunk():
                    if gq:
                        (j,) = gq.pop(0)
                        nc.tensor.matmul(
                            psc[:, ds(128 * j, 128)],
                            lhsT=g[:, j, :], rhs=sb_idbs,
                            start=False, stop=False,
                        )

                # ssan products: half-width psum (deeper rotation), full-width pr
                prods = []
                for i in range(5):
                    eng = PROD[i]
                    rr = 0 if (i % 2 == 0) else 64
                    pr = prp.tile([128, 1024], BF16, tag=f"pr{i}", name="pr")
                    for half in range(2):
                        sl = ds(512 * half, 512)
                        pb = putil.tile([128, 512], F32, tag="u", name="pb")
                        nc.tensor.matmul(
                            pb,
                            lhsT=qw_sb[h][rr:rr + 64, i, ds(128 * t, 128)],
                            rhs=kd[h][rr:rr + 64, sl],
                            start=True, stop=True, tile_position=(rr, 0),
                        )
                        g_chunk()
                        if eng in ("A", "B"):
                            be = bep.tile([128, 512], BF16, tag=f"be{half}", name="be")
                            if use_absb:
                                nc.scalar.activation(
                                    be, pb, AF.Identity,
                                    bias=sb_absb[:, ds(3 * i + h, 1)],
                                )
                            else:
                                nc.scalar.activation(be, pb, AF.Copy)
                            e = nc.vector if eng == "A" else nc.gpsimd
                            e.tensor_tensor(pr[:, sl], be, st[:, i, sl], OP.mult)
                        else:
                            # DVE one-pass (psum read + multiply); Pool can't
                            # read PSUM so "P" is not a valid direct path.
                            if use_absb:
                                nc.vector.scalar_tensor_tensor(
                                    pr[:, sl], in0=pb,
                                    scalar=sb_absb[:, ds(3 * i + h, 1)],
                                    in1=st[:, i, sl], op0=OP.add, op1=OP.mult,
                                )
                            else:
                                nc.vector.tensor_tensor(pr[:, sl], pb, st[:, i, sl], OP.mult)
                    prods.append(pr)
                while gq:
                    g_chunk()
                d["prods"] = prods

            def stageC1(u):
                d = state[u]
                ordered = [p for i, p in enumerate(d["prods"]) if PROD[i] == "D"] + \
                          [p for i, p in enumerate(d["prods"]) if PROD[i] != "D"]
                direct = ordered[:NJOIN]
                rest = ordered[NJOIN:]
                tree_eng = [nc.gpsimd, nc.vector]
                ti = 0
                while len(rest) >= 2:
                    nrest = []
                    for a, b in zip(rest[::2], rest[1::2]):
                        sm = prp.tile([128, 1024], BF16, tag="sm", name="sm")
                        tree_eng[ti % 2].tensor_tensor(sm, a, b, OP.add)
                        ti += 1
                        nrest.append(sm)
                    if len(rest) % 2:
                        nrest.append(rest[-1])
                    rest = nrest
                d["joins"] = direct + rest

            def stageC(u):
                t, h = divmod(u, 3)
                d = state[u]
                psc, r1 = d["psc"], d["r1"]
                joins = [(r1, sb_idbs)] + [(j, sb_idb) for j in d["joins"]]
                for ji, (j, ident) in enumerate(joins):
                    for half in range(2):
                        sl = ds(512 * half, 512)
                        nc.tensor.matmul(
                            psc[:, sl], lhsT=ident, rhs=j[:, sl],
                            start=False,
                            stop=(ji == len(joins) - 1),
                        )
                probs = prob.tile([128, 1024], BF16, tag="p", name="probs")
                rsum = misc.tile([128, 1], F32, tag="rs", name="rsum")
                nc.scalar.activation(probs, psc, AF.Exp, scale=0.125, accum_out=rsum)
                d["probs"], d["rsum"] = probs, rsum

            def stageD1(u):
                t, h = divmod(u, 3)
                d = state[u]
                probs = d["probs"]
                ptps = putil.tile([128, 1024], BF16, tag="u", name="ptps")
                for j in range(NT):
                    nc.tensor.matmul(
                        ptps[:, ds(128 * j, 128)],
                        lhsT=probs[:, ds(128 * j, 128)], rhs=sb_idb,
                        is_transpose=True,
                        start=(j == 0), stop=(j == NT - 1),
                    )
                ptsb = misc.tile([128, 1024], BF16, tag="ptsb", name="ptsb")
                nc.scalar.activation(ptsb, ptps, AF.Copy)
                d["ptsb"] = ptsb

            def stageD2(u):
                t, h = divmod(u, 3)
                d = state.pop(u)
                ptsb, rsum = d["ptsb"], d["rsum"]
                ctxps = putil.tile([128, 64], F32, tag="u", name="ctxps")
                for j in range(NT):
                    nc.tensor.matmul(
                        ctxps,
                        lhsT=ptsb[:, ds(128 * j, 128)],
                        rhs=vsb[:, j, ds(64 * h, 64)],
                        start=(j == 0), stop=(j == NT - 1),
                    )
                rec = misc.tile([128, 1], F32, tag="rc", name="rec")
                nc.vector.reciprocal(rec, rsum)
                nc.vector.tensor_scalar_mul(osb[:, t, ds(64 * h, 64)], ctxps, rec)
                if h == 2:
                    nc.sync.dma_start(out[t], osb[:, t, :])

            NU = 3 * NT
            for u in range(RB_PF):
                stageDMA(u)
            for k in range(NU + 3):
                if k + RB_PF < NU:
                    stageDMA(k + RB_PF)
                if k < len(weave):
                    emit_window(*weave[k])
                if 0 <= k - 1 < NU:
                    stageC1(k - 1)
                if k < NU:
                    stageA(k)
                    stageB(k)
                if 0 <= k - 1 < NU:
                    stageC(k - 1)
                if 0 <= k - 2 < NU:
                    stageD1(k - 2)
                if 0 <= k - 3 < NU:
                    stageD2(k - 3)

    nc.compile()
    return nc, out


_PROGRAM_CACHE = {}


def kernel(**inputs):
    hs = np.asarray(inputs["hidden_states"], np.float32)
    mask = np.asarray(inputs["attention_mask"], np.float32)
    struct = np.asarray(inputs["struct_matrix"], np.float32)
    Wq = np.asarray(inputs["Wq"], np.float32)
    bq = np.asarray(inputs["bq"], np.float32)
    Wk = np.asarray(inputs["Wk"], np.float32)
    bk = np.asarray(inputs["bk"], np.float32)
    Wv = np.asarray(inputs["Wv"], np.float32)
    bv = np.asarray(inputs["bv"], np.float32)
    E = np.asarray(inputs["dist_emb"], np.float32)
    ssw = np.asarray(inputs["ssan_w"], np.float32)
    absb = np.asarray(inputs["abs_bias"], np.float32)

    bf = ml_dtypes.bfloat16
    use_mask = bool(np.any(mask))
    use_pbias = bool(np.any(bq) or np.any(bk) or np.any(bv))
    use_absb = bool(np.any(absb))

    key = (use_mask, use_pbias, use_absb)
    if key not in _PROGRAM_CACHE:
        _PROGRAM_CACHE[key] = build_program(use_mask, use_pbias, use_absb=use_absb)
    nc, _ = _PROGRAM_CACHE[key]

    f8 = ml_dtypes.float8_e4m3fn
    Epad = np.concatenate([E, np.zeros((1, DH), np.float32)])
    Erev = np.concatenate([E[::-1], np.zeros((1, DH), np.float32)])
    ert_half = np.ascontiguousarray(Erev.T) * 16.0   # [64, 2048]
    et_half = np.ascontiguousarray(Epad.T) * 16.0
    ert_np = np.ascontiguousarray(
        np.stack([ert_half[0:32], ert_half[32:64]], axis=1)).astype(f8)
    et_np = np.ascontiguousarray(
        np.stack([et_half[0:32], et_half[32:64]], axis=1)).astype(f8)
    idb_np = np.eye(128, dtype=np.float32).astype(bf)
    idbs_np = (np.eye(128, dtype=np.float32) / 16.0).astype(bf)

    in_maps = []
    for c in range(8):
        b = c // 4
        h0 = 3 * (c % 4)
        hsT = hs[b].T
        m = {
            "hsT": np.ascontiguousarray(
                hsT.reshape(NCHUNK, 128, 1024).transpose(1, 0, 2)
            ).astype(bf),
            "wq": np.ascontiguousarray(
                Wq[:, h0 * 64:(h0 + 3) * 64]
                .reshape(NCHUNK, 128, 192).transpose(1, 0, 2)
            ).astype(bf),
            "wk": np.ascontiguousarray(
                Wk[:, h0 * 64:(h0 + 3) * 64]
                .reshape(NCHUNK, 128, 192).transpose(1, 0, 2)
            ).astype(bf),
            "wv": np.ascontiguousarray(
                Wv[:, h0 * 64:(h0 + 3) * 64]
                .reshape(NCHUNK, 128, 192).transpose(1, 0, 2)
            ).astype(bf),
            "ert": ert_np,
            "et": et_np,
            "ssw": np.ascontiguousarray(
                (ssw[:, h0:h0 + 3] * 8.0).transpose(2, 0, 1, 3)
            ).astype(bf),
            "struct": np.ascontiguousarray(
                struct[:, b, 0].reshape(5, NT, 128, 1024).transpose(1, 2, 0, 3)
            ).astype(bf),
            "absb": np.concatenate(
                [absb[:, h0:h0 + 3].reshape(1, 15) * 8.0,
                 np.zeros((1, 1), np.float32)], 1
            ),
            "idb": idb_np,
            "idbs": idbs_np,
        }
        if use_mask:
            m["maskv"] = (mask[b, 0, 0].reshape(1, 1024) * 8.0).astype(bf)
            m["onesv"] = np.ones((1, 128), np.float32).astype(bf)
        if use_pbias:
            m["bqv"] = bq[h0 * 64:(h0 + 3) * 64].reshape(1, 192).astype(bf)
            m["bkv"] = bk[h0 * 64:(h0 + 3) * 64].reshape(1, 192).astype(bf)
            m["bvv"] = bv[h0 * 64:(h0 + 3) * 64].reshape(1, 192).astype(bf)
            m["onesL"] = np.ones((1, 1024), np.float32).astype(bf)
        in_maps.append(m)

    res = run_bass_kernel_spmd(nc, in_maps, core_ids=list(range(8)))
    outs = [r["out"] for r in res.results]

    full = np.zeros((B, L, D), np.float32)
    for c in range(8):
        b = c // 4
        h0 = 3 * (c % 4)
        o = np.asarray(outs[c], np.float32).reshape(L, 192)
        for j in range(3):
            full[b, :, (h0 + j) * 64:(h0 + j + 1) * 64] = o[:, j * 64:(j + 1) * 64]
    return full
